# revision 24
# baseline (speedup 1.0000x reference)
"""MoE expert FFN (swiglu) kernel for 8 trn2 NeuronCores.

Expert parallelism: 8 experts, one per core. Each core computes, for its
expert e:
    h   = x_e @ w1_e            # [2048, 2048] @ [2048, 2816]
    act = silu(h[:, :1408]) * h[:, 1408:]
    out = act @ w2_e            # [2048, 1408] @ [1408, 2048]

Tokens arrive pre-sorted by expert with equal counts (2048/expert), so
sharding is a static slice and the gather is a concat. No collectives.

v5 (vs v1 baseline at 511us; v2 hit a 2.0GHz P0 downclock at 591us;
v3/v4 hung on HW - suspects: N=512 warmup matmuls / ACT-ring stores):
  - w1 host-packed into per-pair column slabs [j, p, (s, k, c)] loaded in
    consumption order: pair j waits only its own 1MB slab instead of the
    whole 11.5MB (v1 stalled the PE ~28us at chunk-0 start).
  - x host-packed per chunk [c, p, (k, t)]: one contiguous 2MB DMA per
    chunk; chunk 0 in 4 pieces so the first matmul starts sooner.
  - Ring plan: ACT ring carries ONLY x0 (the startup-critical load);
    SP ring carries w1 pairs then x1/w2 (needed from ~90us) then out
    stores; SWDGE carries x2/x3 whose buffer-free waits would block a
    HWDGE ring. v2 streamed x1+w2 concurrently with the w1 pairs and
    starved them (8.7us stall).
  - NO PE warmup burst: v2's 64 back-to-back N=64 warmup matmuls are the
    prime suspect for tripping the sticky P0 power state (2.4->2.0GHz,
    216->259ns per matmul) that made v2 slower than v1 despite 40us less
    stall. Cold-start HAM cost (~1.7us) is cheap insurance.
  - out stored bf16 (cast during PSUM->SBUF drain), halving store bytes.

Device layout (bf16 compute, fp32 PSUM accumulation):
  mm1: inter[f, t] psum tiles; lhsT = w1 slab slice (stationary),
       rhs = x[k, t] (moving, N=512) -> no on-device transpose anywhere.
  swiglu: act_j = silu(ps_a)*ps_b via ACT(Silu) + DVE mul -> bf16 SBUF.
  mm2: out[t, h]; lhsT = act 128-col slice, rhs = w2[f, h] (moving 512).

PE floor: 2112 matmuls x 215.8ns = 456us at 2.4GHz.
"""

import os
import sys

sys.path.insert(0, "/opt/trn_rl_repo")

import numpy as np
import ml_dtypes

E = 8             # experts == cores
T_TOTAL = 16384
H = 2048
F = 1408
F2 = 2 * F        # 2816
TPC = T_TOTAL // E  # 2048 tokens per core
CHUNK = 512
NCH = TPC // CHUNK          # 4 chunks
KH = H // 128               # 16 contraction tiles for mm1
NF = F // 128               # 11 f-blocks per half (a / b)
NT = CHUNK // 128           # 4 m-tiles per chunk in mm2
NHO = H // 512              # 4 output column blocks
XW = KH * CHUNK             # 8192 x-chunk tile width (k, t)
W1W = 2 * KH * 128          # 4096 w1 pair tile width (s, k, c)
ACT_FN = "Silu"             # swap to "Copy" for CoreSim (no Silu there)

_CACHE = {}

# Optional knobs read by test.py (not used by the grading harness).
TRACE = os.environ.get("BASS_TRACE_KERNEL", "0") == "1"
LAST = {}


def _build():
    from concourse import bacc, tile, mybir

    bf16 = mybir.dt.bfloat16
    f32 = mybir.dt.float32
    SILU = getattr(mybir.ActivationFunctionType, ACT_FN)

    # Bacc (not plain Bass): its lowering pipeline splits multi-sem waits
    # into EventSemaphore pairs — TRN2 allows at most 1 wait per instruction.
    nc = bacc.Bacc()
    x_d = nc.declare_dram_parameter("x", [NCH * 128, XW], bf16, isOutput=False)
    w1_d = nc.declare_dram_parameter("w1", [NF * 128, W1W], bf16, isOutput=False)
    w2_d = nc.declare_dram_parameter("w2", [128, NF * H], bf16, isOutput=False)
    out_d = nc.declare_dram_parameter("out", [TPC, H], bf16, isOutput=True)

    with tile.TileContext(nc) as tc:
        with (
            tc.tile_pool(name="w1p", bufs=1) as w1p,
            tc.tile_pool(name="w2p", bufs=1) as w2p,
            tc.tile_pool(name="xp", bufs=2) as xp,
            tc.tile_pool(name="actp", bufs=1) as actp,
            tc.tile_pool(name="tmpp", bufs=2) as tmpp,
            tc.tile_pool(name="outp", bufs=2) as outp,
            tc.tile_pool(name="psp", bufs=8, space="PSUM") as psp,
        ):
            # --- x chunk tiles; chunk 0 in 4 ascending pieces on the
            # otherwise-empty ACT ring (startup-critical: pair-0's k-sweep
            # consumes k-ascending; a small first piece starts the PE ~2us
            # sooner, receipts of later bigger pieces pipeline).
            x_t = []
            for c in range(NCH):
                x_t.append(xp.tile([128, XW], bf16, tag="xc", name=f"x_{c}"))
            for lo, hi in ((0, 1024), (1024, 2048), (2048, 5120), (5120, 8192)):
                nc.scalar.dma_start(
                    out=x_t[0][:, lo:hi],
                    in_=x_d[0:128, lo:hi],
                )

            # --- w1 pair slabs on the SP ring, in consumption order. Slabs
            # are (k, s, c)-ordered so a k-halfslab (all 8 k-tiles of both
            # swiglu halves) is one contiguous 512KB piece. Pairs 0-3 load
            # as half-slabs matching the chunk-0 half-sweep order below;
            # pair 0's first half in two pieces for an earlier first matmul.
            w1_t = []
            for j in range(NF):
                w1_t.append(w1p.tile([128, W1W], bf16, tag=f"w1_{j}", name=f"w1_{j}"))
            nc.sync.dma_start(out=w1_t[0][:, 0:1024], in_=w1_d[0:128, 0:1024])
            nc.sync.dma_start(out=w1_t[0][:, 1024:2048], in_=w1_d[0:128, 1024:2048])
            for j in range(1, 4):
                nc.sync.dma_start(
                    out=w1_t[j][:, 0:2048], in_=w1_d[j * 128 : (j + 1) * 128, 0:2048]
                )
            for j in range(0, 4):
                nc.sync.dma_start(
                    out=w1_t[j][:, 2048:4096],
                    in_=w1_d[j * 128 : (j + 1) * 128, 2048:4096],
                )
            for j in range(4, NF):
                nc.sync.dma_start(out=w1_t[j][:], in_=w1_d[j * 128 : (j + 1) * 128, :])

            # --- x1 and w2 also on the SP ring AFTER all w1 pairs: they are
            # not needed until ~90us, and streaming them early (v2) starved
            # the w1 pair stream the PE was waiting on. w2 is one wide tile
            # loaded by a single 5.8MB DMA: one completion semaphore instead
            # of 11, so mm2's first chunk pays one PE wait instead of 11.
            nc.sync.dma_start(out=x_t[1][:], in_=x_d[128:256, :])
            w2t = w2p.tile([128, NF * H], bf16, tag="w2", name="w2t")
            nc.sync.dma_start(out=w2t[:], in_=w2_d[:, :])

            # --- x2/x3 on SWDGE: their buffer-free waits (x0/x1 reuse)
            # would block a HWDGE ring; gpsimd has nothing else queued.
            nc.gpsimd.dma_start(out=x_t[2][:], in_=x_d[256:384, :])
            nc.gpsimd.dma_start(out=x_t[3][:], in_=x_d[384:512, :])

            def mm1_pair_k(ps_a, ps_b, j, xc, k, start, stop):
                # slab cols are (k, s, c): a at k*256, b at k*256+128
                nc.tensor.matmul(
                    ps_a[:],
                    w1_t[j][:, k * 256 : k * 256 + 128],
                    xc[:, k * CHUNK : (k + 1) * CHUNK],
                    start=start,
                    stop=stop,
                )
                nc.tensor.matmul(
                    ps_b[:],
                    w1_t[j][:, k * 256 + 128 : k * 256 + 256],
                    xc[:, k * CHUNK : (k + 1) * CHUNK],
                    start=start,
                    stop=stop,
                )

            def swiglu(j, ps_a, ps_b, act_t):
                tmp = tmpp.tile([128, CHUNK], f32, tag="tmp", name=f"tmp_{j}")
                nc.scalar.activation(tmp[:], ps_a[:], SILU)
                a = actp.tile([128, CHUNK], bf16, tag=f"act_{j}", name=f"a_{j}")
                act_t.append(a)
                nc.vector.tensor_mul(a[:], tmp[:], ps_b[:])

            for c in range(NCH):
                xc = x_t[c]
                act_t = []

                if c == 0:
                    # Chunk 0: pairs 0-3 in half-k sweeps across all 8 PSUM
                    # banks. Each 512KB half-slab buys 3.5us of matmuls and
                    # x0's first half is reused 4x before the second half is
                    # needed, so the startup demand (~220GB/s) stays under
                    # the DMA ceiling instead of pinning pair-0 completion
                    # at the 3.5MB / 358GB/s serial-load bound.
                    ps04 = []
                    for j in range(4):
                        ps_a = psp.tile([128, CHUNK], f32, tag="ps", name=f"psa0_{j}")
                        ps_b = psp.tile([128, CHUNK], f32, tag="ps", name=f"psb0_{j}")
                        ps04.append((ps_a, ps_b))
                        for k in range(KH // 2):
                            mm1_pair_k(ps_a, ps_b, j, xc, k, k == 0, False)
                    for j in range(4):
                        ps_a, ps_b = ps04[j]
                        for k in range(KH // 2, KH):
                            mm1_pair_k(ps_a, ps_b, j, xc, k, False, k == KH - 1)
                        swiglu(j, ps_a, ps_b, act_t)
                    first = 4
                else:
                    first = 0

                # mm1 + swiglu, one (a, b) f-block pair at a time.
                for j in range(first, NF):
                    ps_a = psp.tile([128, CHUNK], f32, tag="ps")
                    ps_b = psp.tile([128, CHUNK], f32, tag="ps")
                    for k in range(KH):
                        mm1_pair_k(ps_a, ps_b, j, xc, k,
                                   k == 0, k == KH - 1)
                    swiglu(j, ps_a, ps_b, act_t)

                # mm2: out[t, h] for this chunk; bf16 staging.
                for m in range(NT):
                    last = (c == NCH - 1 and m == NT - 1)
                    po = [
                        psp.tile([128, 512], f32, tag="ps", name=f"po_{c}_{m}_{n}")
                        for n in range(NHO)
                    ]
                    osb = outp.tile([128, H], bf16, tag="osb")
                    r0 = c * CHUNK + m * 128
                    if not last:
                        # n-inner: the 4 psum tiles retire within ~850ns of
                        # each other; one copy batch + one 512KB store.
                        for k in range(NF):
                            lhsT = act_t[k][:, m * 128 : (m + 1) * 128]
                            for n in range(NHO):
                                nc.tensor.matmul(
                                    po[n][:],
                                    lhsT,
                                    w2t[:, k * H + n * 512 : k * H + (n + 1) * 512],
                                    start=(k == 0),
                                    stop=(k == NF - 1),
                                )
                        for n in range(NHO):
                            nc.scalar.copy(osb[:, n * 512 : (n + 1) * 512], po[n][:])
                        nc.sync.dma_start(out=out_d[r0 : r0 + 128, :], in_=osb[:])
                    else:
                        # last m-tile n-outer: po[n] retires after 11 matmuls,
                        # so its copy+128KB store overlap po[n+1]'s matmuls —
                        # the kernel tail is one copy + one small store
                        # instead of 4 copies + a 512KB store.
                        for n in range(NHO):
                            for k in range(NF):
                                nc.tensor.matmul(
                                    po[n][:],
                                    act_t[k][:, m * 128 : (m + 1) * 128],
                                    w2t[:, k * H + n * 512 : k * H + (n + 1) * 512],
                                    start=(k == 0),
                                    stop=(k == NF - 1),
                                )
                            nc.scalar.copy(osb[:, n * 512 : (n + 1) * 512], po[n][:])
                            nc.sync.dma_start(
                                out=out_d[r0 : r0 + 128, n * 512 : (n + 1) * 512],
                                in_=osb[:, n * 512 : (n + 1) * 512],
                            )
    if not nc.is_finalized():
        nc.finalize()  # Bacc.finalize runs the lowering pipeline (sem split, alloc_regs)
    return nc


def _get_nc():
    if "nc" not in _CACHE:
        _CACHE["nc"] = _build()
    return _CACHE["nc"]


def kernel(permuted_hidden_states, num_tokens_per_expert, w1, w2):
    from concourse.bass_utils import run_bass_kernel_spmd

    x = np.asarray(permuted_hidden_states, dtype=np.float32)
    w1 = np.asarray(w1, dtype=np.float32)
    w2 = np.asarray(w2, dtype=np.float32)
    ntpe = np.asarray(num_tokens_per_expert)
    assert x.shape == (T_TOTAL, H) and w1.shape == (E, H, F2) and w2.shape == (E, F, H)
    # Reference semantics rely on the static equal split.
    assert np.all(ntpe == TPC), f"expected equal {TPC}-token splits, got {ntpe}"

    bf = ml_dtypes.bfloat16
    in_maps = []
    for e in range(E):
        xe = x[e * TPC : (e + 1) * TPC].astype(bf)
        # x: [c, p, (k, t)] — chunk-major, partition = hidden row within k-tile
        xr = np.ascontiguousarray(
            xe.reshape(NCH, CHUNK, KH, 128).transpose(0, 3, 2, 1)
        ).reshape(NCH * 128, XW)
        w1e = w1[e].astype(bf)
        # w1: [j, p, (k, s, c)] — pair-major slabs, k-major within, with
        # the swiglu a/b 128-col blocks interleaved per k so a k-halfslab
        # is contiguous.
        A = w1e[:, :F].reshape(KH, 128, NF, 128).transpose(2, 1, 0, 3)
        B = w1e[:, F:].reshape(KH, 128, NF, 128).transpose(2, 1, 0, 3)
        w1r = np.ascontiguousarray(
            np.stack([A, B], axis=3)
        ).reshape(NF * 128, W1W)
        # w2: [p, (k, c)] — one wide tile, partition = row within k-tile
        w2r = np.ascontiguousarray(
            w2[e].astype(bf).reshape(NF, 128, H).transpose(1, 0, 2)
        ).reshape(128, NF * H)
        in_maps.append(
            {
                "x": xr,
                "w1": w1r,
                "w2": w2r,
            }
        )

    nc = _get_nc()
    res = run_bass_kernel_spmd(nc, in_maps, list(range(E)), trace=TRACE)
    LAST["exec_time_ns"] = res.exec_time_ns
    LAST["mean_exec_time_ns"] = res.mean_exec_time_ns
    LAST["profile_json"] = res.profile_json
    out = np.concatenate([res.results[i]["out"] for i in range(E)], axis=0)
    return np.ascontiguousarray(out.astype(np.float32))
